# revision 1
# baseline (speedup 1.0000x reference)
"""Trainium2 Bass kernel for nn_AttentionLayer (masked attention pooling).

Reference math (per batch row b):
    pq      = tanh(qe @ Wq.T + bq).reshape(D, H)
    v_b     = pq @ Wr.T                         # collapse H before item
    s       = item_b @ v_b                      # (N,)
    att     = exp(s) * mask   (softmax shift-invariant; |s|<~50 so no max sub)
    denom   = sum(att); if denom_shifted < 1e-7: denom += exp(smax)
    out_b   = (att @ item_b) / denom            # (D,)

Layout strategy: item is host-transposed to [N, BS, D] bf16 so each on-chip
tile is [n-partitions, (row, d)-free].  In this layout:
  - scores run on DVE: tmp = item * v_dense, then an in-block halving tree
    over d.  v_dense is the per-group v strip replicated to all partitions
    via a DRAM bounce (SBUF->DRAM, then a partition-stride-0 DRAM read at
    full HBM rate; an SBUF-source broadcast is single-port-bound and ~10x
    slower).
  - pooling runs on TensorE as block-diagonal matmuls: per 4-row block,
    lhsT = att[:, 4b:4b+4] (M=4, nearly-free weight load), rhs = the
    4 rows' item slice [n, 512].  Out [4, 512] lands in PSUM at a
    32-aligned base (tile_position); the useful diagonal [4, 128] strips
    are pulled out by partition-strided ACT copies (in/out partitions
    match, so the op is lane-legal) and a small composite-AP DMA
    unscrambles row order.
  - denom[r] via matmul(lhsT=att_tile, rhs=ones) -> psum [R, 1] row-major.
  - smax (needed for the reference's denom<1e-7 fallback, which DOES
    trigger on this data) via PE transposes of the [n, R] score tiles.

Distribution: pure data-parallel over batch across 8 cores (256 rows each).
"""

import sys

if "/opt/trn_rl_repo" not in sys.path:
    sys.path.insert(0, "/opt/trn_rl_repo")

from contextlib import ExitStack

import numpy as np

import concourse.bass as bass
import concourse.bacc as bacc
import concourse.tile as tile
from concourse import masks, mybir

B, N, D, H = 2048, 200, 128, 8
NCORES = 8
BS = B // NCORES          # 256 batch rows per core
P = 128                   # partitions
DH = D * H                # 1024
R = 64                    # rows per group
NG = BS // R              # 4 groups per core
N0 = 128                  # n-tile 0 partitions
N1 = N - N0               # n-tile 1 partitions (72)
NBK = R // 16             # psum banks per group (4): 16 rows per bank

F32 = mybir.dt.float32
BF16 = mybir.dt.bfloat16
AX = mybir.AxisListType
OP = mybir.AluOpType
ACT = mybir.ActivationFunctionType

_CACHE = {}


def build_module() -> bass.Bass:
    nc = bacc.Bacc("TRN2", target_bir_lowering=False)

    item_t = nc.declare_dram_parameter("item_t", [N, BS * D], BF16, isOutput=False)
    maskT = nc.declare_dram_parameter("maskT", [N, BS], BF16, isOutput=False)
    bq = nc.declare_dram_parameter("bq", [1, DH], BF16, isOutput=False)
    wr_rep_in = nc.declare_dram_parameter("Wr_rep", [1, DH], BF16, isOutput=False)
    wqT_in = nc.declare_dram_parameter("WqT", [D, DH], BF16, isOutput=False)
    qeT_in = nc.declare_dram_parameter("qeT", [D, BS], BF16, isOutput=False)
    out = nc.declare_dram_parameter("out", [BS, D], F32, isOutput=True)

    with tile.TileContext(nc) as tc, ExitStack() as ctx:
        const = ctx.enter_context(tc.tile_pool(name="const", bufs=1))
        psA = ctx.enter_context(tc.tile_pool(name="psA", bufs=2, space="PSUM"))
        psP = ctx.enter_context(tc.tile_pool(name="psP", bufs=NBK, space="PSUM"))
        psC = ctx.enter_context(tc.tile_pool(name="psC", bufs=2, space="PSUM"))
        dram = ctx.enter_context(tc.tile_pool(name="dram", bufs=1, space="DRAM"))
        items = ctx.enter_context(tc.tile_pool(name="items", bufs=3))
        vden = ctx.enter_context(tc.tile_pool(name="vden", bufs=2))
        # bufs=1: DVE program order already serializes tmp reuse group-to-group
        tmps = ctx.enter_context(tc.tile_pool(name="tmps", bufs=1))
        work = ctx.enter_context(tc.tile_pool(name="work", bufs=2))
        small = ctx.enter_context(tc.tile_pool(name="small", bufs=4))

        # ---- preamble DMAs: projection operands first (they gate the v
        # chain), then mask, then the first item tiles.
        wqT = const.tile([P, DH], BF16)
        nc.sync.dma_start(wqT[:], wqT_in[:])
        qeT_all = const.tile([P, BS], BF16)
        nc.sync.dma_start(qeT_all[:], qeT_in[:])
        bq_sb = const.tile([1, DH], BF16)
        nc.sync.dma_start(bq_sb[:], bq[:])
        wr_rep = const.tile([P, DH], BF16)
        nc.sync.dma_start(wr_rep[:], wr_rep_in[0:1, :].to_broadcast([P, DH]))
        maskT_sb = const.tile([P, BS], BF16)
        nc.sync.dma_start(maskT_sb[:], maskT[0:N0, :])
        maskT1_sb = const.tile([N1, BS], BF16)
        nc.sync.dma_start(maskT1_sb[:], maskT[N0:N, :])

        ones1 = const.tile([1, P], BF16)
        nc.vector.memset(ones1[:], 1.0)
        onesK = const.tile([P, 1], BF16)
        nc.vector.memset(onesK[:], 1.0)
        ident = const.tile([P, P], F32)
        masks.make_identity(nc, ident[:])

        it_tiles = {}

        def issue_item_dma(g):
            it0 = items.tile([N0, R * D], BF16, tag="it0")
            src0 = item_t[0:N0, g * R * D:(g + 1) * R * D]
            hw = R * D // 2
            nc.sync.dma_start(it0[:, 0:hw], src0[:, 0:hw])
            nc.sync.dma_start(it0[:, hw:], src0[:, hw:])
            it1 = items.tile([N1, R * D], BF16, tag="it1")
            src1 = item_t[N0:N, g * R * D:(g + 1) * R * D]
            nc.sync.dma_start(it1[:, 0:hw], src1[:, 0:hw])
            nc.sync.dma_start(it1[:, hw:], src1[:, hw:])
            it_tiles[g] = (it0, it1)

        issue_item_dma(0)

        # ---- query projection -> v (row-major [row, D]), per 128-row half
        vbs = []
        for half in range(2):
            rows = slice(half * P, (half + 1) * P)
            pqt = work.tile([P, DH], BF16, tag="pqt")
            for j in range(2):
                js = slice(j * 512, (j + 1) * 512)
                pq_ps = psA.tile([P, 512], F32, tag="pq")
                nc.tensor.matmul(
                    pq_ps[:], qeT_all[:, rows], wqT[:, js], start=True, stop=False)
                nc.tensor.matmul(
                    pq_ps[:], ones1[:], bq_sb[:, js], start=False, stop=True)
                nc.scalar.activation(pqt[:, js], pq_ps[:], ACT.Tanh)
            tmpv = work.tile([P, DH], BF16, tag="tmpv")
            tmpv3 = tmpv[:].rearrange("p (d h) -> p d h", h=H)
            nc.vector.tensor_tensor(tmpv[:], pqt[:], wr_rep[:], OP.mult)
            v_f32 = work.tile([P, D], F32, tag="vf")
            nc.vector.tensor_reduce(v_f32[:], tmpv3, axis=AX.X, op=OP.add)
            vb = work.tile([P, D], BF16, tag="vb")
            nc.vector.tensor_copy(out=vb[:], in_=v_f32[:])
            vbs.append(vb)

        # v rows -> flat DRAM scratch (row-major), for per-group
        # partition-stride-0 broadcast reads at full HBM rate.
        vdram = dram.tile([1, BS * D], BF16)
        for half in range(2):
            nc.sync.dma_start(
                vdram[0:1, half * P * D:(half + 1) * P * D], vbs[half][:])
        # per-group 1/denom strips bounce through DRAM the same way
        invdram = dram.tile([1, BS], F32)

        tails = []

        def s_phase(g, ti, it, np_, mk, vd):
            tmp = tmps.tile([np_, R * D], BF16, tag=f"tmp{ti}")
            nc.vector.tensor_tensor(tmp[:], it[:], vd[0:np_, :], OP.mult)
            t3 = tmp[:].rearrange("p (r d) -> p r d", d=D)
            dd = D
            while dd > 8:
                dd //= 2
                nc.vector.tensor_tensor(
                    t3[:, :, 0:dd], t3[:, :, 0:dd], t3[:, :, dd:2 * dd], OP.add)
            s = work.tile([np_, R], F32, tag=f"s{ti}")
            nc.vector.tensor_reduce(s[:], t3[:, :, 0:8], axis=AX.X, op=OP.add)
            e = work.tile([np_, R], BF16, tag=f"e{ti}")
            nc.scalar.activation(e[:], s[:], ACT.Exp)
            att = work.tile([np_, R], BF16, tag=f"att{ti}")
            nc.vector.tensor_tensor(
                att[:], e[:], mk[0:np_, g * R:(g + 1) * R], OP.mult)
            return s, att

        for g in range(NG):
            for gn in (g, g + 1, g + 2):
                if gn < NG and gn not in it_tiles:
                    issue_item_dma(gn)
            it0, it1 = it_tiles[g]

            vd = vden.tile([P, R * D], BF16, tag="vd")
            nc.sync.dma_start(
                vd[:],
                vdram[0:1, g * R * D:(g + 1) * R * D].to_broadcast([P, R * D]))

            comb = psC.tile([R, N0 + N1 + 1], F32, tag="comb")
            s0T = comb[:, 0:N0]
            s1T = comb[:, N0:N0 + N1]
            dn = comb[:, N0 + N1:N0 + N1 + 1]
            pbs = [psP.tile([P, 512], F32, tag="pb", name=f"pb{g}_{Bk}")
                   for Bk in range(NBK)]

            # ---- tile-0 scores, then transposes; tile-1 scores; pooling
            s0, att0 = s_phase(g, 0, it0, N0, maskT_sb, vd)
            nc.tensor.transpose(s0T, s0[:], ident[:])
            s1, att1 = s_phase(g, 1, it1, N1, maskT1_sb, vd)
            nc.tensor.transpose(s1T, s1[:], ident[0:N1, 0:N1])
            nc.tensor.matmul(dn, att0[:], onesK[:], start=True, stop=False)
            nc.tensor.matmul(dn, att1[:], onesK[0:N1, :], start=False, stop=True)
            for b in range(4 * NBK):
                o = pbs[b // 4][32 * (b % 4):32 * (b % 4) + 4, :]
                nc.tensor.matmul(
                    o, att0[:, 4 * b:4 * b + 4], it0[:, 4 * b * D:(4 * b + 4) * D],
                    start=True, stop=False, tile_position=(0, 32 * (b % 4)))
                nc.tensor.matmul(
                    o, att1[:, 4 * b:4 * b + 4], it1[:, 4 * b * D:(4 * b + 4) * D],
                    start=False, stop=True, tile_position=(0, 32 * (b % 4)))

            # ---- tail: diagonal extraction, row unscramble, smax fallback,
            # normalize, store.  Emitted one group late so the ACT/DVE never
            # stall on this group's PE results.
            def make_tail(g, s0T, s1T, dn, pbs):
                def tail():
                    # smax + the denom<1e-7 fallback -> inv, bounced through
                    # DRAM to an [16, 512]-arranged form matching acc2.
                    smax = small.tile([R, 1], F32, tag="sm")
                    nc.vector.tensor_reduce(smax[:], s0T, axis=AX.X, op=OP.max)
                    sm1 = small.tile([R, 1], F32, tag="sm1")
                    nc.vector.tensor_reduce(sm1[:], s1T, axis=AX.X, op=OP.max)
                    nc.vector.tensor_tensor(smax[:], smax[:], sm1[:], OP.max)
                    es = small.tile([R, 1], F32, tag="es")
                    nc.scalar.activation(es[:], smax[:], ACT.Exp)
                    thr = small.tile([R, 1], F32, tag="th")
                    nc.vector.tensor_scalar(thr[:], es[:], 1e-7, None, OP.mult)
                    dn2 = small.tile([R, 1], F32, tag="dn2")
                    nc.vector.scalar_tensor_tensor(
                        dn2[:], dn, thr[:], es[:], op0=OP.is_lt, op1=OP.mult)
                    nc.vector.tensor_tensor(dn2[:], dn2[:], dn, OP.add)
                    inv = small.tile([R, 1], F32, tag="iv")
                    nc.vector.reciprocal(inv[:], dn2[:])
                    nc.sync.dma_start(invdram[0:1, g * R:(g + 1) * R], inv[:])
                    # inva[p, Bk] = inv[16Bk+p]: four tiny contiguous reads
                    inva = work.tile([16, NBK], F32, tag="inva")
                    for Bk in range(NBK):
                        nc.sync.dma_start(
                            inva[:, Bk:Bk + 1],
                            invdram[0:1, g * R + 16 * Bk:g * R + 16 * Bk + 16])

                    # extraction: bulk-copy each psum bank to sbuf on ACT,
                    # then one clean-strided DMA per diagonal lane c into
                    # acc2[4j+c, Bk*128+d] (bank Bk row 16Bk+4j+c lives at
                    # partition 32j+c, elem c*D).
                    acc2 = work.tile([16, NBK * D], F32, tag="acc2")
                    ebig = work.tile([P, NBK * 512], F32, tag="ebig")
                    for Bk in range(NBK):
                        nc.scalar.copy(
                            ebig[:, Bk * 512:(Bk + 1) * 512], pbs[Bk][:])
                    FW = NBK * 512
                    for c in range(4):
                        src = bass.AP(
                            tensor=ebig[:].tensor,
                            offset=ebig[:].offset + c * FW + c * D,
                            ap=[[32 * FW, 4], [512, NBK], [1, D]])
                        dst = bass.AP(
                            tensor=acc2[:].tensor,
                            offset=acc2[:].offset + c * NBK * D,
                            ap=[[4 * NBK * D, 4], [D, NBK], [1, D]])
                        nc.sync.dma_start(dst, src)
                    # normalize in the packed layout (per-partition scalar
                    # per bank block), then store with a row-unscrambling
                    # DRAM access pattern
                    for Bk in range(NBK):
                        nc.vector.tensor_scalar(
                            acc2[:, Bk * D:(Bk + 1) * D],
                            acc2[:, Bk * D:(Bk + 1) * D],
                            inva[:, Bk:Bk + 1], None, OP.mult)
                    dst_o = bass.AP(
                        tensor=out[:].tensor,
                        offset=out[:].offset + g * R * D,
                        ap=[[D, 16], [16 * D, NBK], [1, D]])
                    src_o = bass.AP(
                        tensor=acc2[:].tensor, offset=acc2[:].offset,
                        ap=[[NBK * D, 16], [D, NBK], [1, D]])
                    nc.sync.dma_start(dst_o, src_o)
                return tail
            tails.append(make_tail(g, s0T, s1T, dn, pbs))
            if g >= 1:
                tails[g - 1]()
                tails[g - 1] = None
        tails[NG - 1]()

    nc.compile()
    return nc


def _get_module() -> bass.Bass:
    if "nc" not in _CACHE:
        _CACHE["nc"] = build_module()
    return _CACHE["nc"]


def make_in_maps(item_embedding, query_embedding, mask, Wq, bq, Wr):
    import ml_dtypes

    bf16 = ml_dtypes.bfloat16
    item = np.asarray(item_embedding, dtype=np.float32)
    qe = np.asarray(query_embedding, dtype=np.float32)
    mk = np.asarray(mask).reshape(B, N)
    wq = np.asarray(Wq, dtype=np.float32)
    bqr = np.ascontiguousarray(bq.reshape(1, DH)).astype(bf16)
    wr = np.asarray(Wr, dtype=np.float32)
    wr_rep = np.ascontiguousarray(np.tile(wr.reshape(1, H), (1, D))).astype(bf16)
    wqT = np.ascontiguousarray(wq.T).astype(bf16)
    in_maps = []
    for i in range(NCORES):
        r = slice(i * BS, (i + 1) * BS)
        # [BS, N, D] -> bf16 -> [N, BS, D] contiguous -> [N, BS*D]
        it = np.ascontiguousarray(
            item[r].astype(bf16).transpose(1, 0, 2)).reshape(N, BS * D)
        mt = np.ascontiguousarray(mk[r].T.astype(bf16))
        in_maps.append({
            "item_t": it,
            "maskT": mt,
            "bq": bqr,
            "Wr_rep": wr_rep,
            "WqT": wqT,
            "qeT": np.ascontiguousarray(qe[r].T.astype(bf16)),
        })
    return in_maps


def kernel(item_embedding, query_embedding, mask, Wq, bq, Wr):
    from concourse.bass_utils import run_bass_kernel_spmd

    nc = _get_module()
    in_maps = make_in_maps(item_embedding, query_embedding, mask, Wq, bq, Wr)
    last_err = None
    for attempt in range(3):
        try:
            res = run_bass_kernel_spmd(
                nc, in_maps, core_ids=list(range(NCORES)),
                **_CACHE.get("run_kwargs", {})
            )
            break
        except Exception as e:  # transient NRT_EXEC_UNIT_UNRECOVERABLE flakes
            last_err = e
    else:
        raise last_err
    _CACHE["last_results"] = res
    return np.concatenate([res.results[i]["out"] for i in range(NCORES)], axis=0)

